# revision 6
# baseline (speedup 1.0000x reference)
"""Grouped-expert SwiGLU (MoE) kernel for Trainium2, expert-parallel over 8 cores.

Per core (one expert):
    g = x @ W_gate          [T, DOUT]
    u = x @ W_down          [T, DOUT]
    h = silu(g) * u
    out = h @ W_up          [T, DIN]

All inputs are pre-cast to bf16 and pre-laid-out on the host so the device
does no transposes and no input casts — the PE runs a dense LDW+MM stream at
the bf16 roofline (~216 ns per [128x128]x[128x512] matmul):
  x_t    [S1, KC, P, NS]  xT chunks: x_t[s,k,p,n] = x[s*NS+n, k*P+p]
  gate_t [JC, P, DIN]     per-j panels: gate_t[j,p,k*P+n] = Wg[k*P+p, j*P+n]
  down_t [JC, P, DIN]     same layout as gate_t
  up_t   [JC, P, DIN]     up_t[j,p,c] = Wu[j*P+p, c]
phase 1: hT[j] = silu(Wg[:,j].T @ xT) * (Wd[:,j].T @ xT)   [dout, tokens]
phase 2: out[m,:] = sum_j hT[j][:,m].T @ Wu[j,:]           [tokens, din]
Matmuls in bf16 with fp32 PSUM accumulation.
"""

import numpy as np
import ml_dtypes

import concourse.bacc as bacc
import concourse.mybir as mybir
from concourse.tile import TileContext
from concourse.bass_utils import run_bass_kernel_spmd

F32 = mybir.dt.float32
BF16 = mybir.dt.bfloat16
SILU = mybir.ActivationFunctionType.Silu
SIGMOID = mybir.ActivationFunctionType.Sigmoid
COPY = mybir.ActivationFunctionType.Copy

E = 8
T, DIN, DOUT = 2048, 2048, 1408
P = 128
NS = 512
KC = DIN // P   # 16 contraction chunks (din)
JC = DOUT // P  # 11 dout blocks
MC = T // P     # 16 token blocks
S1 = T // NS    # 4 token strips
S2 = DIN // NS  # 4 din strips


def build_program(sim_safe=False):
    nc = bacc.Bacc(target_bir_lowering=False, trn_type="TRN2")
    xt = nc.dram_tensor("x_t", [S1, KC, P, NS], BF16, kind="ExternalInput")
    wg = nc.dram_tensor("gate_t", [JC, P, DIN], BF16, kind="ExternalInput")
    wd = nc.dram_tensor("down_t", [JC, P, DIN], BF16, kind="ExternalInput")
    wu = nc.dram_tensor("up_t", [JC, P, DIN], BF16, kind="ExternalInput")
    out = nc.dram_tensor("out", [T, DIN], F32, kind="ExternalOutput")

    with TileContext(nc) as tc:
        with tc.tile_pool(name="persist", bufs=1) as persist:
            xts = [[persist.tile([P, NS], BF16, tag=f"xt0_{k}",
                                 name=f"xt0_{k}")
                    for k in range(KC)] for _ in range(1)]
            xts3 = [persist.tile([P, KC, NS], BF16, tag=f"xt3_{s}",
                                 name=f"xt3_{s}")
                    for s in range(1, S1)]
            hT = [persist.tile([P, T], BF16, tag=f"hT{j}", name=f"hT{j}")
                  for j in range(JC)]
            wub = [persist.tile([P, DIN], BF16, tag=f"wub{j}", name=f"wub{j}")
                   for j in range(JC)]

            with tc.tile_pool(name="wstage", bufs=2) as wstage, \
                 tc.tile_pool(name="silu", bufs=3) as silu_pool, \
                 tc.tile_pool(name="ostage", bufs=4) as ostage, \
                 tc.tile_pool(name="p1", bufs=2, space="PSUM") as p1, \
                 tc.tile_pool(name="p2", bufs=4, space="PSUM") as p2:

                # DMA emission follows consumption order: j0 panels first,
                # then strip-0 chunks (fine-grained so the k-loop starts on
                # chunk 0), then strips 1-3 as single big DMAs (dispatch on
                # the Sync ring is ~650ns/DMA and serializes — it, not HBM,
                # bounds early supply).  x strips go on the Scalar ring so
                # both rings dispatch in parallel.  wub staging is deferred
                # out of the j0 critical window.
                def x_rhs(s, k):
                    if s == 0:
                        return xts[0][k]
                    return xts3[s - 1][:, k, :]

                # ---- phase 1: hT[j] = silu(gT) * uT ----
                for j in range(JC):
                    wgp = wstage.tile([P, DIN], BF16, tag="wgp", name=f"wgp{j}")
                    wdp = wstage.tile([P, DIN], BF16, tag="wdp", name=f"wdp{j}")
                    nc.sync.dma_start(out=wgp, in_=wg.ap()[j])
                    nc.sync.dma_start(out=wdp, in_=wd.ap()[j])
                    if j == 0:
                        for k in range(KC):
                            nc.scalar.dma_start(out=xts[0][k],
                                                in_=xt.ap()[0, k])
                        for s in range(1, S1):
                            nc.scalar.dma_start(
                                out=xts3[s - 1],
                                in_=xt.ap()[s].rearrange("k p n -> p k n"))
                    for s in range(S1):
                        pg = p1.tile([P, NS], F32, tag="pg", name="pg")
                        pu = p1.tile([P, NS], F32, tag="pu", name="pu")
                        for k in range(KC):
                            nc.tensor.matmul(
                                pg, lhsT=wgp[:, k * P:(k + 1) * P],
                                rhs=x_rhs(s, k),
                                start=(k == 0), stop=(k == KC - 1))
                        for k in range(KC):
                            nc.tensor.matmul(
                                pu, lhsT=wdp[:, k * P:(k + 1) * P],
                                rhs=x_rhs(s, k),
                                start=(k == 0), stop=(k == KC - 1))
                        sl = silu_pool.tile([P, NS], BF16, tag="sl", name="sl")
                        if sim_safe:
                            # CoreSim has no Silu; silu(g) = g * sigmoid(g)
                            nc.scalar.activation(sl, pg, SIGMOID)
                            nc.vector.tensor_mul(out=sl, in0=sl, in1=pg)
                        else:
                            nc.scalar.activation(sl, pg, SILU)
                        nc.vector.tensor_mul(out=hT[j][:, s * NS:(s + 1) * NS],
                                             in0=sl, in1=pu)
                    # stage phase-2 weights after this j's critical DMAs
                    nc.sync.dma_start(out=wub[j], in_=wu.ap()[j])

                # ---- phase 2: out = hT.T @ Wu ----
                for m in range(MC):
                    for n in range(S2):
                        dsl = slice(n * NS, (n + 1) * NS)
                        po = p2.tile([P, NS], F32, tag="po", name="po")
                        for j in range(JC):
                            nc.tensor.matmul(
                                po, lhsT=hT[j][:, m * P:(m + 1) * P],
                                rhs=wub[j][:, dsl],
                                start=(j == 0), stop=(j == JC - 1))
                        ot = ostage.tile([P, NS], F32, tag="ot", name="ot")
                        if (m * S2 + n) % 2 == 0:
                            nc.scalar.activation(ot, po, COPY)
                        else:
                            nc.vector.tensor_copy(out=ot, in_=po)
                        nc.sync.dma_start(
                            out=out.ap()[m * P:(m + 1) * P, dsl], in_=ot)

    nc.finalize()
    return nc


_BF = ml_dtypes.bfloat16


def make_in_maps(x, gate_proj, down_proj, up_proj):
    maps = []
    for e in range(E):
        xtb = x[e].T.astype(_BF)  # [DIN, T]
        xtb = np.ascontiguousarray(
            xtb.reshape(KC, P, S1, NS).transpose(2, 0, 1, 3))
        gtb = np.ascontiguousarray(
            gate_proj[e].astype(_BF).reshape(KC, P, JC, P)
            .transpose(2, 1, 0, 3)).reshape(JC, P, DIN)
        dtb = np.ascontiguousarray(
            down_proj[e].astype(_BF).reshape(KC, P, JC, P)
            .transpose(2, 1, 0, 3)).reshape(JC, P, DIN)
        utb = np.ascontiguousarray(up_proj[e].astype(_BF)).reshape(JC, P, DIN)
        maps.append({"x_t": xtb, "gate_t": gtb, "down_t": dtb, "up_t": utb})
    return maps


_program = None


def kernel(x, gate_proj, down_proj, up_proj):
    global _program
    if _program is None:
        _program = build_program()
    in_maps = make_in_maps(
        np.asarray(x, dtype=np.float32),
        np.asarray(gate_proj, dtype=np.float32),
        np.asarray(down_proj, dtype=np.float32),
        np.asarray(up_proj, dtype=np.float32),
    )
    res = run_bass_kernel_spmd(_program, in_maps, list(range(E)))
    return np.stack([res.results[e]["out"] for e in range(E)], axis=0)


# revision 10
# speedup vs baseline: 1.0385x; 1.0385x over previous
"""Grouped-expert SwiGLU (MoE) kernel for Trainium2, expert-parallel over 8 cores.

Per core (one expert):
    g = x @ W_gate          [T, DOUT]
    u = x @ W_down          [T, DOUT]
    h = silu(g) * u
    out = h @ W_up          [T, DIN]

All inputs are pre-cast to bf16 and pre-laid-out on the host so the device
does no transposes and no input casts — the PE runs a dense LDW+MM stream at
the bf16 roofline (~216 ns per [128x128]x[128x512] matmul):
  x_t    [S1, KC, P, NS]  xT chunks: x_t[s,k,p,n] = x[s*NS+n, k*P+p]
  gate_t [JC, P, DIN]     per-j panels: gate_t[j,p,k*P+n] = Wg[k*P+p, j*P+n]
  down_t [JC, P, DIN]     same layout as gate_t
  up_t   [JC, P, DIN]     up_t[j,p,c] = Wu[j*P+p, c]
phase 1: hT[j] = silu(Wg[:,j].T @ xT) * (Wd[:,j].T @ xT)   [dout, tokens]
phase 2: out[m,:] = sum_j hT[j][:,m].T @ Wu[j,:]           [tokens, din]
Matmuls in bf16 with fp32 PSUM accumulation.
"""

import numpy as np
import ml_dtypes

import concourse.bacc as bacc
import concourse.mybir as mybir
from concourse.tile import TileContext
from concourse.bass_utils import run_bass_kernel_spmd

F32 = mybir.dt.float32
BF16 = mybir.dt.bfloat16
SILU = mybir.ActivationFunctionType.Silu
SIGMOID = mybir.ActivationFunctionType.Sigmoid
COPY = mybir.ActivationFunctionType.Copy

E = 8
T, DIN, DOUT = 2048, 2048, 1408
P = 128
NS = 512
KC = DIN // P   # 16 contraction chunks (din)
JC = DOUT // P  # 11 dout blocks
MC = T // P     # 16 token blocks
S1 = T // NS    # 4 token strips
S2 = DIN // NS  # 4 din strips


def build_program(sim_safe=False):
    nc = bacc.Bacc(target_bir_lowering=False, trn_type="TRN2")
    xt = nc.dram_tensor("x_t", [S1, KC, P, NS], BF16, kind="ExternalInput")
    wg = nc.dram_tensor("gate_t", [JC, P, DIN], BF16, kind="ExternalInput")
    wd = nc.dram_tensor("down_t", [JC, P, DIN], BF16, kind="ExternalInput")
    wu = nc.dram_tensor("up_t", [JC, P, DIN], BF16, kind="ExternalInput")
    out = nc.dram_tensor("out", [T, DIN], F32, kind="ExternalOutput")

    with TileContext(nc) as tc:
        with tc.tile_pool(name="persist", bufs=1) as persist:
            xts0 = [persist.tile([P, 2, NS], BF16, tag=f"xt0_{c}",
                                 name=f"xt0_{c}")
                    for c in range(KC // 2)]
            xts3 = [persist.tile([P, KC, NS], BF16, tag=f"xt3_{s}",
                                 name=f"xt3_{s}")
                    for s in range(1, S1)]
            hT = [persist.tile([P, T], BF16, tag=f"hT{j}", name=f"hT{j}")
                  for j in range(JC)]
            wub = [persist.tile([P, DIN], BF16, tag=f"wub{j}", name=f"wub{j}")
                   for j in range(JC)]

            with tc.tile_pool(name="wstage", bufs=2) as wstage, \
                 tc.tile_pool(name="silu", bufs=3) as silu_pool, \
                 tc.tile_pool(name="ostage", bufs=4) as ostage, \
                 tc.tile_pool(name="p1", bufs=2, space="PSUM") as p1, \
                 tc.tile_pool(name="p2", bufs=4, space="PSUM") as p2:

                # DMA emission follows consumption order: j0 panels first,
                # then strip-0 chunks (fine-grained so the k-loop starts on
                # chunk 0), then strips 1-3 as single big DMAs (dispatch on
                # the Sync ring is ~650ns/DMA and serializes — it, not HBM,
                # bounds early supply).  x strips go on the Scalar ring so
                # both rings dispatch in parallel.  wub staging is deferred
                # out of the j0 critical window.
                def x_rhs(s, k):
                    if s == 0:
                        return xts0[k // 2][:, k % 2, :]
                    return xts3[s - 1][:, k, :]

                # ---- phase 1: hT[j] = silu(gT) * uT ----
                for j in range(JC):
                    wgp = wstage.tile([P, DIN], BF16, tag="wgp", name=f"wgp{j}")
                    wdp = wstage.tile([P, DIN], BF16, tag="wdp", name=f"wdp{j}")
                    nc.sync.dma_start(out=wgp, in_=wg.ap()[j])
                    nc.sync.dma_start(out=wdp, in_=wd.ap()[j])
                    if j == 0:
                        # strip 0 in 256KB chunks on the Scalar ring
                        for c in range(KC // 2):
                            nc.scalar.dma_start(
                                out=xts0[c],
                                in_=xt.ap()[0, 2 * c:2 * c + 2]
                                .rearrange("k p n -> p k n"))
                        # strips 1-3 in 512KB quarters on the Sync ring
                        for s in range(1, S1):
                            for q in range(4):
                                nc.sync.dma_start(
                                    out=xts3[s - 1][:, 4 * q:4 * q + 4, :],
                                    in_=xt.ap()[s, 4 * q:4 * q + 4]
                                    .rearrange("k p n -> p k n"))
                    for s in range(S1):
                        pg = p1.tile([P, NS], F32, tag="pg", name="pg")
                        pu = p1.tile([P, NS], F32, tag="pu", name="pu")
                        for k in range(KC):
                            nc.tensor.matmul(
                                pg, lhsT=wgp[:, k * P:(k + 1) * P],
                                rhs=x_rhs(s, k),
                                start=(k == 0), stop=(k == KC - 1))
                        for k in range(KC):
                            nc.tensor.matmul(
                                pu, lhsT=wdp[:, k * P:(k + 1) * P],
                                rhs=x_rhs(s, k),
                                start=(k == 0), stop=(k == KC - 1))
                        sl = silu_pool.tile([P, NS], BF16, tag="sl", name="sl")
                        if sim_safe:
                            # CoreSim has no Silu; silu(g) = g * sigmoid(g)
                            nc.scalar.activation(sl, pg, SIGMOID)
                            nc.vector.tensor_mul(out=sl, in0=sl, in1=pg)
                        else:
                            nc.scalar.activation(sl, pg, SILU)
                        nc.vector.tensor_mul(out=hT[j][:, s * NS:(s + 1) * NS],
                                             in0=sl, in1=pu)

                # stage phase-2 weights; the Sync ring reaches these right
                # after the phase-1 panels, well before phase 2 needs them
                for j in range(JC):
                    nc.sync.dma_start(out=wub[j], in_=wu.ap()[j])

                # ---- phase 2: out = hT.T @ Wu ----
                for m in range(MC):
                    for n in range(S2):
                        dsl = slice(n * NS, (n + 1) * NS)
                        po = p2.tile([P, NS], F32, tag="po", name="po")
                        for j in range(JC):
                            nc.tensor.matmul(
                                po, lhsT=hT[j][:, m * P:(m + 1) * P],
                                rhs=wub[j][:, dsl],
                                start=(j == 0), stop=(j == JC - 1))
                        ot = ostage.tile([P, NS], F32, tag="ot", name="ot")
                        if (m * S2 + n) % 2 == 0:
                            nc.scalar.activation(ot, po, COPY)
                        else:
                            nc.vector.tensor_copy(out=ot, in_=po)
                        nc.sync.dma_start(
                            out=out.ap()[m * P:(m + 1) * P, dsl], in_=ot)

    nc.finalize()
    return nc


_BF = ml_dtypes.bfloat16


def make_in_maps(x, gate_proj, down_proj, up_proj):
    maps = []
    for e in range(E):
        xtb = x[e].T.astype(_BF)  # [DIN, T]
        xtb = np.ascontiguousarray(
            xtb.reshape(KC, P, S1, NS).transpose(2, 0, 1, 3))
        gtb = np.ascontiguousarray(
            gate_proj[e].astype(_BF).reshape(KC, P, JC, P)
            .transpose(2, 1, 0, 3)).reshape(JC, P, DIN)
        dtb = np.ascontiguousarray(
            down_proj[e].astype(_BF).reshape(KC, P, JC, P)
            .transpose(2, 1, 0, 3)).reshape(JC, P, DIN)
        utb = np.ascontiguousarray(up_proj[e].astype(_BF)).reshape(JC, P, DIN)
        maps.append({"x_t": xtb, "gate_t": gtb, "down_t": dtb, "up_t": utb})
    return maps


_program = None


def kernel(x, gate_proj, down_proj, up_proj):
    global _program
    if _program is None:
        _program = build_program()
    in_maps = make_in_maps(
        np.asarray(x, dtype=np.float32),
        np.asarray(gate_proj, dtype=np.float32),
        np.asarray(down_proj, dtype=np.float32),
        np.asarray(up_proj, dtype=np.float32),
    )
    res = run_bass_kernel_spmd(_program, in_maps, list(range(E)))
    return np.stack([res.results[e]["out"] for e in range(E)], axis=0)
